# revision 9
# baseline (speedup 1.0000x reference)
"""MoE feed-forward (16 experts, top-2, capacity 1280) on 8 Trainium2 NeuronCores.

Strategy (expert-parallel, matching the sharding hint):
  - Router runs on host in fp32 (replicating the reference's routing math
    bit-closely; top-2 selection margin on this distribution is ~7e-5, far
    above fp32 noise, so selection is stable).
  - Each core owns 2 experts. x (bf16) is replicated to every core; each core
    gathers the rows for its experts' capacity slots (indirect DMA), runs the
    two expert FFNs in bf16 (fp32 PSUM accumulation, exact-erf Gelu on ACT),
    and indirect-scatters weighted output rows into an AllToAll send buffer
    bucketed by destination core.
  - One AllToAll moves every expert-output row to the core owning its token.
    Each core then combines (gate-weighted sum of its tokens' two rows, plus
    the gate-weighted expert biases precomputed on host) and writes its
    1024-token slice of the output.
"""

import numpy as np
import ml_dtypes

import concourse.bass as bass
import concourse.mybir as mybir
import concourse.tile as tile
from concourse import bacc
from concourse.bass_utils import run_bass_kernel_spmd
from concourse.masks import make_identity

# Problem shapes (hardcoded per the contract)
NCORES = 8
E, TOPK = 16, 2
D, DFF = 1024, 4096
B, S = 4, 2048
T = B * S                # 8192 tokens
TPC = T // NCORES        # 1024 tokens per core
EPC = E // NCORES        # 2 experts per core
CAP = 1280               # ceil(1.25 * T * TOPK / E)
BUCKET = 384             # a2a rows per (owner core -> dest core); actual max ~304
NROWS = NCORES * BUCKET  # 3072 rows in each a2a buffer
SENTINEL = NROWS         # > bounds_check -> scatter silently skipped

KD = D // 128            # 8 k-tiles over d
NF = DFF // 128          # 32 tiles over dff
SLOT_CHUNKS = [(0, 512), (512, 512), (1024, 256)]

BF16 = mybir.dt.bfloat16
F32 = mybir.dt.float32
I32 = mybir.dt.int32

_CACHE = {}


def _build_nc(reps=1):
    """Build the SPMD program. reps>1 repeats the whole kernel body inside one
    NEFF (used only for timing: per-iteration delta cancels dispatch cost)."""
    key = ("nc", reps)
    if key in _CACHE:
        return _CACHE[key]
    nc = bacc.Bacc("TRN2", target_bir_lowering=False, debug=False, num_devices=NCORES)

    xb = nc.dram_tensor("xb", [T, D], BF16, kind="ExternalInput").ap()
    w1b = nc.dram_tensor("w1b", [EPC, D, DFF], BF16, kind="ExternalInput").ap()
    w2b = nc.dram_tensor("w2b", [EPC, DFF, D], BF16, kind="ExternalInput").ap()
    b1t = nc.dram_tensor("b1t", [EPC, 128, NF], F32, kind="ExternalInput").ap()
    tok = nc.dram_tensor("tok", [EPC, CAP, 1], I32, kind="ExternalInput").ap()
    dst = nc.dram_tensor("dst", [EPC, CAP, 1], I32, kind="ExternalInput").ap()
    cr0 = nc.dram_tensor("cr0", [TPC, 1], I32, kind="ExternalInput").ap()
    cr1 = nc.dram_tensor("cr1", [TPC, 1], I32, kind="ExternalInput").ap()
    cw0 = nc.dram_tensor("cw0", [TPC, 1], F32, kind="ExternalInput").ap()
    cw1 = nc.dram_tensor("cw1", [TPC, 1], F32, kind="ExternalInput").ap()
    cb = nc.dram_tensor("cb", [TPC, D], F32, kind="ExternalInput").ap()
    out = nc.dram_tensor("out", [TPC, D], F32, kind="ExternalOutput").ap()

    with tile.TileContext(nc) as tc:
        with (
            tc.tile_pool(name="const", bufs=1) as constp,
            tc.tile_pool(name="idxp", bufs=4) as idxp,
            tc.tile_pool(name="tps", bufs=2, space="PSUM") as tps,
            tc.tile_pool(name="hps", bufs=2, space="PSUM") as hps,
            tc.tile_pool(name="yps", bufs=2, space="PSUM") as yps,
            tc.tile_pool(name="dram", bufs=1, space="DRAM") as dram,
        ):
            ident = constp.tile([128, 128], BF16)
            make_identity(nc, ident[:])

            a2a_send = dram.tile([NROWS, D], BF16)
            a2a_recv = dram.tile([NROWS, D], BF16)

            for rep in range(reps):
              # Zero the first row of every bucket in the send buffer: dropped
              # assignments read recv row 0, which maps to a sender's row
              # [dst*BUCKET + 0]; zeroing keeps 0-weighted garbage (NaN) out.
              zrow = constp.tile([NCORES, D], BF16, tag="zrow")
              nc.gpsimd.memset(zrow[:], 0.0)
              send_first_rows = bass.AP(
                  a2a_send.tensor, 0, [[BUCKET * D, NCORES], [1, D]]
              )
              nc.sync.dma_start(send_first_rows, zrow[:])

              expert_pools = [
                  tc.tile_pool(name=f"w1p{rep}", bufs=KD),
                  tc.tile_pool(name=f"w2p{rep}", bufs=NF),
                  tc.tile_pool(name=f"b1p{rep}", bufs=EPC),
                  tc.tile_pool(name=f"ptp{rep}", bufs=2),
                  tc.tile_pool(name=f"htp{rep}", bufs=1),
                  tc.tile_pool(name=f"pgp{rep}", bufs=3),
                  tc.tile_pool(name=f"yp{rep}", bufs=3),
              ]
              w1p, w2p, b1p, ptp, htp, pgp, yp = [p.__enter__() for p in expert_pools]
              for l in range(EPC):
                # Expert weights resident in SBUF (streamed from HBM).
                w1_sb = []
                for kd in range(KD):
                    wt = w1p.tile([128, DFF], BF16, tag="w1")
                    nc.sync.dma_start(wt[:], w1b[l, kd * 128:(kd + 1) * 128, :])
                    w1_sb.append(wt)
                w2_sb = []
                for kf in range(NF):
                    wt = w2p.tile([128, D], BF16, tag="w2")
                    nc.sync.dma_start(wt[:], w2b[l, kf * 128:(kf + 1) * 128, :])
                    w2_sb.append(wt)
                b1_sb = b1p.tile([128, NF], F32, tag="b1")
                nc.sync.dma_start(b1_sb[:], b1t[l])

                for c0, cw in SLOT_CHUNKS:
                    ct = cw // 128  # slot tiles in this chunk
                    # --- dispatch: gather x rows, transpose to [d, slots] ---
                    pt = ptp.tile([128, KD, 512], BF16, tag="pt")
                    for st in range(ct):
                        s_lo = c0 + st * 128
                        tok_sb = idxp.tile([128, 1], I32, tag="idx")
                        nc.sync.dma_start(tok_sb[:], tok[l, s_lo:s_lo + 128, :])
                        pg = pgp.tile([128, D], BF16, tag="pg")
                        nc.gpsimd.indirect_dma_start(
                            out=pg[:],
                            out_offset=None,
                            in_=xb[:, :],
                            in_offset=bass.IndirectOffsetOnAxis(ap=tok_sb[:, 0:1], axis=0),
                        )
                        for kd in range(KD):
                            pt_ps = tps.tile([128, 128], BF16, tag="tps")
                            nc.tensor.transpose(
                                pt_ps[:], pg[:, kd * 128:(kd + 1) * 128], ident[:]
                            )
                            nc.vector.tensor_copy(
                                out=pt[:, kd, st * 128:(st + 1) * 128], in_=pt_ps[:]
                            )

                    # --- mm1 + exact gelu: hT[f, slots] = gelu(w1.T @ pt + b1) ---
                    hT = htp.tile([128, NF, 512], BF16, tag="ht")
                    for f in range(NF):
                        hp = hps.tile([128, 512], F32, tag="hps")
                        for kd in range(KD):
                            nc.tensor.matmul(
                                out=hp[:, :cw],
                                lhsT=w1_sb[kd][:, f * 128:(f + 1) * 128],
                                rhs=pt[:, kd, :cw],
                                start=(kd == 0),
                                stop=(kd == KD - 1),
                            )
                        nc.scalar.activation(
                            out=hT[:, f, :cw],
                            in_=hp[:, :cw],
                            func=mybir.ActivationFunctionType.Gelu,
                            bias=b1_sb[:, f:f + 1],
                            scale=1.0,
                        )

                    # --- mm2: y[slots, d] = hT.T @ w2; cast bf16; scatter ---
                    for st in range(ct):
                        s_lo = c0 + st * 128
                        y_sb = yp.tile([128, D], BF16, tag="y")
                        for nd in range(2):
                            yps_t = yps.tile([128, 512], F32, tag="yps")
                            for kf in range(NF):
                                nc.tensor.matmul(
                                    out=yps_t[:],
                                    lhsT=hT[:, kf, st * 128:(st + 1) * 128],
                                    rhs=w2_sb[kf][:, nd * 512:(nd + 1) * 512],
                                    start=(kf == 0),
                                    stop=(kf == NF - 1),
                                )
                            nc.vector.tensor_copy(
                                out=y_sb[:, nd * 512:(nd + 1) * 512], in_=yps_t[:]
                            )
                        dst_sb = idxp.tile([128, 1], I32, tag="idx")
                        nc.sync.dma_start(dst_sb[:], dst[l, s_lo:s_lo + 128, :])
                        nc.gpsimd.indirect_dma_start(
                            out=a2a_send[:, :],
                            out_offset=bass.IndirectOffsetOnAxis(ap=dst_sb[:, 0:1], axis=0),
                            in_=y_sb[:],
                            in_offset=None,
                            bounds_check=NROWS - 1,
                            oob_is_err=False,
                        )

              for p in reversed(expert_pools):
                  p.__exit__(None, None, None)

              # --- all-to-all: rows to token-owner cores ---
              nc.gpsimd.collective_compute(
                  "AllToAll",
                  mybir.AluOpType.bypass,
                  ins=[a2a_send.opt()],
                  outs=[a2a_recv.opt()],
                  replica_groups=[list(range(NCORES))],
              )

              # --- combine: out[t] = w0*row0 + w1*row1 + cb ---
              with tc.tile_pool(name=f"combp{rep}", bufs=3) as combp:
                for tt in range(TPC // 128):
                    t_lo = tt * 128
                    r0i = idxp.tile([128, 1], I32, tag="idx")
                    r1i = idxp.tile([128, 1], I32, tag="idx")
                    nc.sync.dma_start(r0i[:], cr0[t_lo:t_lo + 128, :])
                    nc.sync.dma_start(r1i[:], cr1[t_lo:t_lo + 128, :])
                    r0_sb = combp.tile([128, D], BF16, tag="r0")
                    r1_sb = combp.tile([128, D], BF16, tag="r1")
                    nc.gpsimd.indirect_dma_start(
                        out=r0_sb[:], out_offset=None, in_=a2a_recv[:, :],
                        in_offset=bass.IndirectOffsetOnAxis(ap=r0i[:, 0:1], axis=0),
                    )
                    nc.gpsimd.indirect_dma_start(
                        out=r1_sb[:], out_offset=None, in_=a2a_recv[:, :],
                        in_offset=bass.IndirectOffsetOnAxis(ap=r1i[:, 0:1], axis=0),
                    )
                    w0_sb = idxp.tile([128, 1], F32, tag="w")
                    w1_sb_c = idxp.tile([128, 1], F32, tag="w")
                    nc.sync.dma_start(w0_sb[:], cw0[t_lo:t_lo + 128, :])
                    nc.sync.dma_start(w1_sb_c[:], cw1[t_lo:t_lo + 128, :])
                    cb_sb = combp.tile([128, D], F32, tag="cb")
                    nc.sync.dma_start(cb_sb[:], cb[t_lo:t_lo + 128, :])

                    t1 = combp.tile([128, D], F32, tag="t1")
                    nc.vector.tensor_scalar_mul(t1[:], r1_sb[:], w1_sb_c[:, 0:1])
                    o_sb = combp.tile([128, D], F32, tag="o")
                    nc.vector.tensor_scalar_mul(o_sb[:], r0_sb[:], w0_sb[:, 0:1])
                    nc.vector.tensor_add(o_sb[:], o_sb[:], t1[:])
                    nc.vector.tensor_add(o_sb[:], o_sb[:], cb_sb[:])
                    nc.sync.dma_start(out[t_lo:t_lo + 128, :], o_sb[:])

    nc.compile()
    _CACHE["nc"] = nc
    return nc


def _routing(x2d, router_w, router_b):
    """Replicates the reference's router + capacity logic in fp32 numpy."""
    logits = x2d @ router_w.T + router_b[None, :]           # [T, E] fp32
    idx = np.argsort(-logits, axis=1, kind="stable")[:, :TOPK].astype(np.int32)
    vals = np.take_along_axis(logits, idx, axis=1)
    m = vals.max(axis=1, keepdims=True)
    eg = np.exp(vals - m)
    gates = eg / eg.sum(axis=1, keepdims=True)              # fp32 softmax

    flat_e = idx.reshape(-1)                                # [T*k]
    oh = (flat_e[:, None] == np.arange(E)[None, :]).astype(np.int32)
    pos = np.take_along_axis(np.cumsum(oh, axis=0) - 1, flat_e[:, None], axis=1)[:, 0]
    kept = pos < CAP
    kept2 = kept.reshape(T, TOPK)
    denom = np.sum(gates * kept2, axis=-1, keepdims=True)
    gates = np.where(kept2, gates / (denom + np.float32(1e-9)), np.float32(0.0))
    return idx, gates.astype(np.float32), flat_e, pos, kept


def prepare_in_maps(x, router_w, router_b, w1, b1, w2, b2):
    x = np.asarray(x, dtype=np.float32)
    router_w = np.asarray(router_w, dtype=np.float32)
    router_b = np.asarray(router_b, dtype=np.float32)
    w1 = np.asarray(w1, dtype=np.float32)
    b1 = np.asarray(b1, dtype=np.float32)
    w2 = np.asarray(w2, dtype=np.float32)
    b2 = np.asarray(b2, dtype=np.float32)

    x2d = x.reshape(T, D)
    idx, gates, flat_e, pos, kept = _routing(x2d, router_w, router_b)
    tok_of = np.arange(T * TOPK, dtype=np.int64) // TOPK

    # token_for_slot per expert
    tfs = np.zeros((E, CAP), dtype=np.int32)
    ka = np.nonzero(kept)[0]
    tfs[flat_e[ka], pos[ka]] = tok_of[ka].astype(np.int32)

    # bucket offsets: order of kept assignments within (owner core, dest core)
    off = np.full(T * TOPK, -1, dtype=np.int64)
    cnt = np.zeros((NCORES, NCORES), dtype=np.int64)
    owner = flat_e // EPC
    dest = tok_of // TPC
    for a in ka:
        o, dc = owner[a], dest[a]
        off[a] = cnt[o, dc]
        cnt[o, dc] += 1
    assert cnt.max() <= BUCKET, f"bucket overflow: {cnt.max()} > {BUCKET}"

    # scatter destinations per expert slot
    dst_idx = np.full((E, CAP), SENTINEL, dtype=np.int32)
    dst_idx[flat_e[ka], pos[ka]] = (dest[ka] * BUCKET + off[ka]).astype(np.int32)

    # combine indices/weights per token
    r_all = np.zeros((T, TOPK), dtype=np.int32)
    w_all = np.zeros((T, TOPK), dtype=np.float32)
    kept2 = kept.reshape(T, TOPK)
    a_ids = np.arange(T * TOPK).reshape(T, TOPK)
    r_all[kept2] = (owner[a_ids[kept2]] * BUCKET + off[a_ids[kept2]]).astype(np.int32)
    w_all[kept2] = gates[kept2]

    # gate-weighted expert biases, combined on host
    cb_full = (w_all[:, 0:1] * b2[idx[:, 0]] + w_all[:, 1:2] * b2[idx[:, 1]]).astype(np.float32)

    xb = np.ascontiguousarray(x2d.astype(ml_dtypes.bfloat16))
    in_maps = []
    for c in range(NCORES):
        e0 = c * EPC
        sl = slice(c * TPC, (c + 1) * TPC)
        in_maps.append({
            "xb": xb,
            "w1b": np.ascontiguousarray(w1[e0:e0 + EPC].astype(ml_dtypes.bfloat16)),
            "w2b": np.ascontiguousarray(w2[e0:e0 + EPC].astype(ml_dtypes.bfloat16)),
            "b1t": np.ascontiguousarray(
                b1[e0:e0 + EPC].reshape(EPC, NF, 128).transpose(0, 2, 1)
            ),
            "tok": np.ascontiguousarray(tfs[e0:e0 + EPC, :, None]),
            "dst": np.ascontiguousarray(dst_idx[e0:e0 + EPC, :, None]),
            "cr0": np.ascontiguousarray(r_all[sl, 0:1]),
            "cr1": np.ascontiguousarray(r_all[sl, 1:2]),
            "cw0": np.ascontiguousarray(w_all[sl, 0:1]),
            "cw1": np.ascontiguousarray(w_all[sl, 1:2]),
            "cb": np.ascontiguousarray(cb_full[sl]),
        })
    return in_maps


def assemble_output(results):
    out = np.concatenate([results[c]["out"] for c in range(NCORES)], axis=0)
    return out.reshape(B, S, D).astype(np.float32)


def kernel(x, router_w, router_b, w1, b1, w2, b2):
    in_maps = prepare_in_maps(x, router_w, router_b, w1, b1, w2, b2)
    nc = _build_nc()
    res = run_bass_kernel_spmd(nc, in_maps, list(range(NCORES)))
    return assemble_output(res.results)
